# revision 1
# baseline (speedup 1.0000x reference)
import numpy as np
import jax
import jax.numpy as jnp
from jax import lax

# Tacotron-style decoder, data-parallel over batch across 8 NeuronCores.
# B=64 -> 8 per core; the sequential scan stays local per shard (no collectives).

IN_F = 512
MEL = 80
R = 7
ATT = 128
B = 64
T_ENC = 512
T_MEL = 700
N_CORES = 8

_PARAM_NAMES = [
    'prenet_w1', 'prenet_b1', 'prenet_w2', 'prenet_b2',
    'attrnn_w_ih', 'attrnn_w_hh', 'attrnn_b_ih', 'attrnn_b_hh',
    'q_w', 'inp_w', 'v_w', 'v_b', 'loc_conv', 'loc_w', 'pd_w', 'pd_b',
    'dec1_w_ih', 'dec1_w_hh', 'dec1_b_ih', 'dec1_b_hh',
    'dec2_w_ih', 'dec2_w_hh', 'dec2_b_ih', 'dec2_b_hh',
    'mel_w', 'mel_b', 'stop_w', 'stop_b',
]


def _gru(x, h, w_ih, w_hh, b_ih, b_hh):
    gi = x @ w_ih.T + b_ih
    gh = h @ w_hh.T + b_hh
    ir, iz, inn = jnp.split(gi, 3, axis=-1)
    hr, hz, hn = jnp.split(gh, 3, axis=-1)
    r = jax.nn.sigmoid(ir + hr)
    z = jax.nn.sigmoid(iz + hz)
    return (1.0 - z) * jnp.tanh(inn + r * hn) + z * h


def _decoder_shard(inputs, memory, mask, params):
    (prenet_w1, prenet_b1, prenet_w2, prenet_b2,
     attrnn_w_ih, attrnn_w_hh, attrnn_b_ih, attrnn_b_hh,
     q_w, inp_w, v_w, v_b, loc_conv, loc_w, pd_w, pd_b,
     dec1_w_ih, dec1_w_hh, dec1_b_ih, dec1_b_hh,
     dec2_w_ih, dec2_w_hh, dec2_b_ih, dec2_b_hh,
     mel_w, mel_b, stop_w, stop_b) = params
    Bq, T_enc, _ = inputs.shape
    proc_in = jnp.einsum('btd,ad->bta', inputs, inp_w)
    mem = memory.reshape(Bq, -1, R * MEL).transpose(1, 0, 2)
    xs = jnp.concatenate([jnp.zeros_like(mem[:1]), mem[:-1]], axis=0)

    def step(carry, x):
        h_att, h1, h2, ctx, aw, awc = carry
        m_in = x[:, MEL * (R - 1):]
        p = jax.nn.relu(m_in @ prenet_w1.T + prenet_b1)
        p = jax.nn.relu(p @ prenet_w2.T + prenet_b2)
        h_att = _gru(jnp.concatenate([p, ctx], axis=-1), h_att,
                     attrnn_w_ih, attrnn_w_hh, attrnn_b_ih, attrnn_b_hh)
        pq = h_att @ q_w.T
        loc = jnp.stack([aw, awc], axis=1)
        pl = lax.conv_general_dilated(loc, loc_conv, (1,), [(15, 15)],
                                      dimension_numbers=('NCH', 'OIH', 'NCH'))
        pl = jnp.einsum('bct,ac->bta', pl, loc_w)
        e = (jnp.tanh(pq[:, None, :] + proc_in + pl) @ v_w.T)[..., 0] + v_b
        e = jnp.where(mask, e, -1e9)
        align = jax.nn.softmax(e, axis=-1)
        ctx = jnp.einsum('bt,btd->bd', align, inputs)
        awc = awc + align
        dec = jnp.concatenate([h_att, ctx], axis=-1) @ pd_w.T + pd_b
        h1 = _gru(dec, h1, dec1_w_ih, dec1_w_hh, dec1_b_ih, dec1_b_hh)
        dec = h1 + dec
        h2 = _gru(dec, h2, dec2_w_ih, dec2_w_hh, dec2_b_ih, dec2_b_hh)
        dec = h2 + dec
        out = dec @ mel_w.T + mel_b
        sin = lax.stop_gradient(jnp.concatenate([dec, out], axis=-1))
        stop = (sin @ stop_w.T + stop_b)[:, 0]
        return (h_att, h1, h2, ctx, align, awc), (out, align, stop)

    z = lambda n: jnp.zeros((Bq, n), inputs.dtype)
    zt = jnp.zeros((Bq, T_enc), inputs.dtype)
    carry0 = (z(256), z(256), z(256), z(IN_F), zt, zt)
    _, (outs, atts, stops) = lax.scan(step, carry0, xs)
    outputs = outs.transpose(1, 0, 2).reshape(Bq, -1, MEL).transpose(0, 2, 1)
    return outputs, atts.transpose(1, 0, 2), stops.T


_pmapped = None


def _get_pmapped():
    global _pmapped
    if _pmapped is None:
        _pmapped = jax.pmap(_decoder_shard, axis_name='x',
                            in_axes=(0, 0, 0, None))
    return _pmapped


def kernel(**inputs):
    inp = jnp.asarray(inputs['inputs'], jnp.float32).reshape(
        N_CORES, B // N_CORES, T_ENC, IN_F)
    mem = jnp.asarray(inputs['memory'], jnp.float32).reshape(
        N_CORES, B // N_CORES, T_MEL, MEL)
    mask = jnp.asarray(inputs['mask']).reshape(N_CORES, B // N_CORES, T_ENC)
    params = tuple(jnp.asarray(inputs[n], jnp.float32) for n in _PARAM_NAMES)
    outputs, atts, stops = _get_pmapped()(inp, mem, mask, params)
    outputs = np.asarray(outputs).reshape(B, MEL, T_MEL)
    atts = np.asarray(atts).reshape(B, T_MEL // R, T_ENC)
    stops = np.asarray(stops).reshape(B, T_MEL // R)
    return outputs, atts, stops
